# revision 2
# baseline (speedup 1.0000x reference)
"""GQA causal attention (B=2, S=2048, D=2048, 16 q heads / 4 kv heads, RoPE)
for 8 Trainium2 NeuronCores.

Sharding: core i = (batch b = i//4, kv-head group g = i%4). Each core computes
its group's Q/K/V projections, RoPE, causal attention and the partial output
projection; the host sums the 4 per-group partials per batch.

On-core layout is fully "transposed" (features on partitions):
  xT [D, S], QT/KT [d, S] -> QK scores land as [k, q], softmax runs along k
  (partitions) with the denominator computed by an all-ones matmul, and PV
  accumulates out^T [d, q] directly in PSUM. The final projection contracts
  over the group's 512 head-dims on partitions.
All matmuls use float32r (full PE throughput at moving dim >= 256).
"""

import sys

sys.path.insert(0, "/opt/trn_rl_repo")

import numpy as np
from contextlib import ExitStack

import concourse.bacc as bacc
import concourse.mybir as mybir
import concourse.tile as tile
from concourse.bass_utils import run_bass_kernel_spmd

B, S, DIM = 2, 2048, 2048
N_HEADS, N_KV, HD = 16, 4, 128
HPG = N_HEADS // N_KV      # q heads per kv group
GD = HPG * HD              # 512 = group width
P = 128
NS = S // 512              # 4 s-slices of 512
NC = DIM // P              # 16 contraction chunks of 128
NKT = S // P               # 16 k tiles
F32 = mybir.dt.float32
F32R = mybir.dt.float32r
SCALE = 1.0 / float(np.sqrt(HD))
MASK_NEG = -1.0e5

# consts column layout
C_RT = 0          # [128]  RoPE rotation (R.T)
C_ID = 128        # [128]  identity
C_ONES = 256      # [128]  all-ones
C_MASK = 384      # [4*512] causal band masks, additive
C_COS = 2432      # [2048] cos, repeated x2 along d
C_SIN = 4480      # [2048]
NCONST = 6528

_CACHE = {}


def _build():
    nc = bacc.Bacc()
    xT = nc.dram_tensor("xT", [DIM, S], F32, kind="ExternalInput")
    wqT = nc.dram_tensor("wqT", [DIM, GD], F32, kind="ExternalInput")
    wkT = nc.dram_tensor("wkT", [DIM, HD], F32, kind="ExternalInput")
    wvT = nc.dram_tensor("wvT", [DIM, HD], F32, kind="ExternalInput")
    woT = nc.dram_tensor("woT", [GD, DIM], F32, kind="ExternalInput")
    consts = nc.dram_tensor("consts", [P, NCONST], F32, kind="ExternalInput")
    out = nc.dram_tensor("out", [S, DIM], F32, kind="ExternalOutput")

    EXP = mybir.ActivationFunctionType.Exp

    with tile.TileContext(nc) as tc, ExitStack() as ctx:
        cpool = ctx.enter_context(tc.tile_pool(name="consts", bufs=1))
        persist = ctx.enter_context(tc.tile_pool(name="persist", bufs=1))

        consts_sb = cpool.tile([P, NCONST], F32R, name="consts_sb")
        rt = consts_sb[:, C_RT:C_RT + 128]
        ident = consts_sb[:, C_ID:C_ID + 128].bitcast(F32)
        ones_r = consts_sb[:, C_ONES:C_ONES + 128]
        masks = consts_sb[:, C_MASK:C_MASK + 4 * 512].bitcast(F32).rearrange(
            "p (r q) -> p r q", r=4)
        cosf = consts_sb[:, C_COS:C_COS + S].bitcast(F32)
        sinf = consts_sb[:, C_SIN:C_SIN + S].bitcast(F32)

        wo_sb = persist.tile([P, HPG, DIM], F32R, name="wo_sb")
        q_sb = persist.tile([P, HPG, S], F32R, name="q_sb")
        k_sb = persist.tile([P, S], F32R, name="k_sb")
        v_sb = persist.tile([P, NKT, HD], F32R, name="v_sb")

        # ---- Phase 1: QKV projections + RoPE + V transpose, per s-slice ----
        with ExitStack() as p1:
            wpool = p1.enter_context(tc.tile_pool(name="wqkv", bufs=1))
            xpool = p1.enter_context(tc.tile_pool(name="xs", bufs=2))
            vtpool = p1.enter_context(tc.tile_pool(name="vt", bufs=1))
            tmpp = p1.enter_context(tc.tile_pool(name="ropetmp", bufs=3))
            psA = p1.enter_context(tc.tile_pool(name="psA", bufs=1, space="PSUM"))
            psR = p1.enter_context(tc.tile_pool(name="psR", bufs=1, space="PSUM"))
            psT = p1.enter_context(tc.tile_pool(name="psT", bufs=1, space="PSUM"))

            wq_sb = wpool.tile([P, NC, GD], F32R, name="wq_sb")
            wk_sb = wpool.tile([P, NC, HD], F32R, name="wk_sb")
            wv_sb = wpool.tile([P, NC, HD], F32R, name="wv_sb")
            vt_sb = vtpool.tile([P, S], F32, name="vt_sb")

            def dma_wq_chunk(cc):
                nc.sync.dma_start(
                    out=wq_sb[:, 4 * cc:4 * (cc + 1), :],
                    in_=wqT[512 * cc:512 * (cc + 1), :]
                    .rearrange("(c p) h -> p c h", p=P).bitcast(F32R))

            def dma_xs(xs, j, cc):
                nc.sync.dma_start(
                    out=xs,
                    in_=xT[512 * cc:512 * (cc + 1), 512 * j:512 * (j + 1)]
                    .rearrange("(c p) s -> p c s", p=P).bitcast(F32R))

            for j in range(NS):
                sl = slice(512 * j, 512 * (j + 1))
                ps = [psA.tile([P, 512], F32, name=f"proj{t}") for t in range(6)]
                for cc in range(4):
                    xs = xpool.tile([P, 4, 512], F32R, name="xs")
                    dma_xs(xs, j, cc)
                    if j == 0:
                        # stagger weight loads behind the x stream so the
                        # first matmuls start as early as possible
                        if cc == 0:
                            dma_wq_chunk(0)
                            nc.sync.dma_start(
                                out=wk_sb, in_=wkT[:, :]
                                .rearrange("(c p) h -> p c h", p=P).bitcast(F32R))
                            nc.sync.dma_start(
                                out=wv_sb, in_=wvT[:, :]
                                .rearrange("(c p) h -> p c h", p=P).bitcast(F32R))
                        else:
                            dma_wq_chunk(cc)
                    for c4 in range(4):
                        c = 4 * cc + c4
                        first = c == 0
                        last = c == NC - 1
                        for t in range(HPG):
                            nc.tensor.matmul(
                                ps[t], wq_sb[:, c, 128 * t:128 * (t + 1)],
                                xs[:, c4, :], start=first, stop=last)
                        nc.tensor.matmul(ps[4], wk_sb[:, c, :], xs[:, c4, :],
                                         start=first, stop=last)
                        nc.tensor.matmul(ps[5], wv_sb[:, c, :], xs[:, c4, :],
                                         start=first, stop=last)
                for t in range(HPG):
                    nc.scalar.copy(q_sb[:, t, sl], ps[t])
                nc.scalar.copy(k_sb[:, sl], ps[4])
                nc.scalar.copy(vt_sb[:, sl], ps[5])
                if j == 0:
                    nc.sync.dma_start(out=consts_sb, in_=consts[:, :].bitcast(F32R))
                elif j == 1:
                    nc.sync.dma_start(
                        out=wo_sb,
                        in_=woT[:, :].rearrange("(c p) e -> p c e", p=P).bitcast(F32R))

                # RoPE for this slice (4 q heads + k)
                for t in range(HPG + 1):
                    src = q_sb[:, t, sl] if t < HPG else k_sb[:, sl]
                    rot = psR.tile([P, 512], F32, name="rot")
                    nc.tensor.matmul(rot, rt, src, start=True, stop=True)
                    t1 = tmpp.tile([P, 512], F32, name="t1")
                    t2 = tmpp.tile([P, 512], F32, name="t2")
                    nc.vector.tensor_mul(t1, rot, sinf[:, sl])
                    nc.vector.tensor_mul(t2, src.bitcast(F32), cosf[:, sl])
                    nc.vector.tensor_add(src, t1, t2)

                # V transpose for this slice's 4 k-tiles
                for kt in range(4 * j, 4 * (j + 1)):
                    tr = psT.tile([P, P], F32, name="tr")
                    nc.tensor.transpose(tr, vt_sb[:, P * kt:P * (kt + 1)], ident)
                    nc.scalar.copy(v_sb[:, kt, :], tr)

        # ---- Phase 2: attention (j outer) + interleaved output projection ----
        with ExitStack() as p3:
            ppool = p3.enter_context(tc.tile_pool(name="ptiles", bufs=6))
            bcpool = p3.enter_context(tc.tile_pool(name="bc", bufs=2))
            attnp = p3.enter_context(tc.tile_pool(name="attn", bufs=1))
            outp = p3.enter_context(tc.tile_pool(name="outp", bufs=4))
            psQK = p3.enter_context(tc.tile_pool(name="psQK", bufs=3, space="PSUM"))
            psPV = p3.enter_context(tc.tile_pool(name="psPV", bufs=2, space="PSUM"))
            psDN = p3.enter_context(tc.tile_pool(name="psDN", bufs=2, space="PSUM"))
            psO = p3.enter_context(tc.tile_pool(name="psO", bufs=1, space="PSUM"))

            attn_sb = attnp.tile([P, HPG, S], F32R, name="attn_sb")

            for j in range(NS):
                sl = slice(512 * j, 512 * (j + 1))
                nkt = 4 * (j + 1)
                for h in range(HPG):
                    pv = psPV.tile([P, 512], F32, name="pv")
                    den = psDN.tile([P, 512], F32, name="den")
                    pts = [None] * nkt

                    def score(kt):
                        qk = psQK.tile([P, 512], F32, name="qk")
                        nc.tensor.matmul(qk, k_sb[:, P * kt:P * (kt + 1)],
                                         q_sb[:, h, sl], start=True, stop=True)
                        r = kt - 4 * j
                        if r >= 0:
                            nc.vector.tensor_add(qk, qk, masks[:, r, :])
                        pt = ppool.tile([P, 512], F32R, name="pt")
                        nc.scalar.activation(pt, qk, EXP, scale=SCALE)
                        pts[kt] = pt

                    def accum(kt):
                        nc.tensor.matmul(pv, v_sb[:, kt, :], pts[kt],
                                         start=(kt == 0), stop=(kt == nkt - 1))
                        nc.tensor.matmul(den, ones_r, pts[kt],
                                         start=(kt == 0), stop=(kt == nkt - 1))

                    score(0)
                    for kt in range(1, nkt):
                        score(kt)
                        accum(kt - 1)
                    accum(nkt - 1)

                    rec_sb = bcpool.tile([P, 512], F32, name="rec_sb")
                    nc.vector.reciprocal_approx_fast(rec_sb, den)
                    nc.vector.tensor_mul(attn_sb[:, h, sl], pv, rec_sb)

                # output projection for the 4 s-tiles completed by this slice
                for st in range(4 * j, 4 * (j + 1)):
                    for e in range(NS):
                        ops = psO.tile([P, 512], F32, name="ops")
                        for hc in range(HPG):
                            nc.tensor.matmul(
                                ops, attn_sb[:, hc, P * st:P * (st + 1)],
                                wo_sb[:, hc, 512 * e:512 * (e + 1)],
                                start=(hc == 0), stop=(hc == HPG - 1))
                        osb = outp.tile([P, 512], F32, name="osb")
                        nc.vector.tensor_copy(osb, ops)
                        nc.sync.dma_start(
                            out=out[P * st:P * (st + 1), 512 * e:512 * (e + 1)],
                            in_=osb)

    nc.compile()
    return nc


def _consts_array(freqs_cos, freqs_sin):
    c = np.zeros((P, NCONST), np.float32)
    rt = np.zeros((P, P), np.float32)
    idx = np.arange(0, P, 2)
    rt[idx, idx + 1] = 1.0    # (R.T)[2j, 2j+1] = +1
    rt[idx + 1, idx] = -1.0   # (R.T)[2j+1, 2j] = -1
    c[:, C_RT:C_RT + P] = rt
    c[:, C_ID:C_ID + P] = np.eye(P, dtype=np.float32)
    c[:, C_ONES:C_ONES + P] = 1.0
    ki = np.arange(P)[:, None]
    qi = np.arange(512)[None, :]
    for r in range(4):
        c[:, C_MASK + 512 * r:C_MASK + 512 * (r + 1)] = np.where(
            ki <= qi - P * r, 0.0, MASK_NEG).astype(np.float32)
    c[:, C_COS:C_COS + S] = np.repeat(np.asarray(freqs_cos, np.float32).T, 2, axis=0)
    c[:, C_SIN:C_SIN + S] = np.repeat(np.asarray(freqs_sin, np.float32).T, 2, axis=0)
    return c


def _in_maps(x, wq, wk, wv, wo, freqs_cos, freqs_sin):
    x = np.asarray(x, np.float32)
    wq = np.asarray(wq, np.float32)
    wk = np.asarray(wk, np.float32)
    wv = np.asarray(wv, np.float32)
    wo = np.asarray(wo, np.float32)
    consts = _consts_array(freqs_cos, freqs_sin)
    maps = []
    for core in range(8):
        b, g = divmod(core, 4)
        maps.append({
            "xT": np.ascontiguousarray(x[b].T),
            "wqT": np.ascontiguousarray(wq[GD * g:GD * (g + 1), :].T),
            "wkT": np.ascontiguousarray(wk[HD * g:HD * (g + 1), :].T),
            "wvT": np.ascontiguousarray(wv[HD * g:HD * (g + 1), :].T),
            "woT": np.ascontiguousarray(wo[:, GD * g:GD * (g + 1)].T),
            "consts": consts,
        })
    return maps


def _get_nc():
    if "nc" not in _CACHE:
        _CACHE["nc"] = _build()
    return _CACHE["nc"]


def _run(in_maps, trace=False):
    return run_bass_kernel_spmd(_get_nc(), in_maps, core_ids=list(range(8)),
                                trace=trace)


def kernel(x, wq, wk, wv, wo, freqs_cos, freqs_sin):
    res = _run(_in_maps(x, wq, wk, wv, wo, freqs_cos, freqs_sin))
    out = np.zeros((B, S, DIM), np.float32)
    for core in range(8):
        b = core // 4
        out[b] += res.results[core]["out"]
    return out
